# revision 15
# baseline (speedup 1.0000x reference)
"""DeepSeek-V3-style MoE gate (nn_MoEGate) on 8 Trainium2 NeuronCores.

Data-parallel: token dim (16384) sharded 8 ways; the [256, 7168] gate weight
is replicated, host-split into bf16 hi/lo halves (w = w_hi + w_lo exactly to
~2^-17 relative) and shipped pre-transposed so the contraction dim lands on
SBUF partitions.

Per core (2048 tokens, 16 tiles of 128):
  - hs tile DMA'd fp32, transposed on PE (fp32 transpose-mode, 4 k-tiles per
    PSUM bank), then split PSUM->SBUF into bf16 hi (ACT rounding copy) and
    bf16 lo (DVE subtract) — hi+lo represents hs to ~2^-17.
  - logits accumulate over 3 bf16 matmul passes at 1 cyc/row:
    hi@w_hi + hi@w_lo + lo@w_hi  (the dropped lo@w_lo term is ~2^-30).
    This reproduces fp32 routing decisions (7/131072 index flips on the
    fixed seed) at 1/4 the fp32 matmul cost.
  - sigmoid on ACT straight out of PSUM, DeepSeek group-limited top-k
    routing on DVE (Max8 / MaxIndex8), gather of unbiased scores via
    iota-compare scalar_tensor_tensor with fused row-reduce.

This walrus build only accepts ONE semaphore wait per instruction, so the
kernel follows a strict sync discipline: cross-engine waits are arranged so
each instruction's single wait transitively implies every other dependency
(ACT hi-copy waits PE, DVE lo-TT waits ACT which implies PE, matmuls wait
the hi/lo producers, the tr bank-WAR waits the DVE lo-TT which implies the
ACT hi-copy). Fresh DMA lanes are absorbed by 1x1 dummy matmuls on PE /
1-element touches on DVE+ACT ordered before the real consumers, and the
kernel tail funnels every leaf through single-wait stores on SP.
"""

import numpy as np
import ml_dtypes
from contextlib import ExitStack

import concourse.bass as bass
import concourse.tile as tile
import concourse.mybir as mybir
from concourse.bass_utils import run_bass_kernel_spmd
from concourse.tile import add_dep_helper

# problem constants (hardcoded per contract)
S_FULL = 16384
H = 7168
E = 256
N_CORES = 8
SL = S_FULL // N_CORES        # 2048 tokens per core
P = 128                       # partitions
HT = H // P                   # 56 k-tiles
ST = SL // P                  # 16 token-tiles per core
NC = 14                       # k-chunks per tile (4 k-tiles each)
KC = 4                        # k-tiles per chunk (one PSUM bank, 512 fp32)
G, GS, TOP_K = 8, 32, 8
TOPK_GROUP = 4
ROUTED_SCALING = 2.5
EPS = 1e-20

f32 = mybir.dt.float32
bf16 = mybir.dt.bfloat16
u32 = mybir.dt.uint32
i32 = mybir.dt.int32


def _dep(a, b, sync=True, reason="dep"):
    if a is None or b is None:
        return
    add_dep_helper(a.ins if hasattr(a, "ins") else a,
                   b.ins if hasattr(b, "ins") else b, sync=sync, reason=reason)


def _funnel(nc, insts, junk_tile):
    """Serialize kernel-tail dependencies through real SP store instructions
    (NOP waits are dropped at lowering; TENSOR_STORE emits a real wait)."""
    for n, inst in enumerate(x for x in insts if x is not None):
        st = nc.sync.store(junk_tile[0:1, n:n + 1], 0)
        _dep(st, inst, sync=True, reason="tail funnel")


def build_program():
    nc = bass.Bass("TRN2", target_bir_lowering=False, debug=False,
                   num_devices=N_CORES)
    hs = nc.dram_tensor("hs", [SL, H], f32, kind="ExternalInput").ap()
    whi = nc.dram_tensor("whi", [P, HT, E], bf16, kind="ExternalInput").ap()
    wlo = nc.dram_tensor("wlo", [P, HT, E], bf16, kind="ExternalInput").ap()
    bias = nc.dram_tensor("bias", [P, E], f32, kind="ExternalInput").ap()
    iota = nc.dram_tensor("iota", [P, E], f32, kind="ExternalInput").ap()
    iden = nc.dram_tensor("iden", [P, P], f32, kind="ExternalInput").ap()
    o_w = nc.dram_tensor("o_w", [SL, TOP_K], f32, kind="ExternalOutput").ap()
    o_i = nc.dram_tensor("o_i", [SL, TOP_K], i32, kind="ExternalOutput").ap()

    with tile.TileContext(nc) as tc, ExitStack() as ctx:
        const = ctx.enter_context(tc.tile_pool(name="const", bufs=1))
        hstp = ctx.enter_context(tc.tile_pool(name="hstp", bufs=2))
        hTp = ctx.enter_context(tc.tile_pool(name="hTp", bufs=2))
        sco = ctx.enter_context(tc.tile_pool(name="sco", bufs=2))
        lps = ctx.enter_context(tc.tile_pool(name="lps", bufs=2, space="PSUM"))
        tps = ctx.enter_context(tc.tile_pool(name="tps", bufs=4, space="PSUM"))
        dps = ctx.enter_context(tc.tile_pool(name="dps", bufs=1, space="PSUM"))

        # Startup: tile-0 hs arrives in two halves at full bandwidth (ACT
        # ring); an SP register load gates the big weight DMAs behind the
        # first half so transposes start ~5us in and whi lands just as the
        # first matmul sweep needs it.
        hst0 = hstp.tile([P, H], f32, tag="hst")
        HH = H // 2
        d_hs0a = nc.scalar.dma_start(out=hst0[:, 0:HH], in_=hs[0:P, 0:HH])
        d_hs0b = nc.scalar.dma_start(out=hst0[:, HH:H], in_=hs[0:P, HH:H])
        iden_sb = const.tile([P, P], f32)
        nc.sync.dma_start(out=iden_sb, in_=iden)
        bias_sb = const.tile([P, E], f32)
        nc.sync.dma_start(out=bias_sb, in_=bias)
        iota_sb = const.tile([P, E], f32)
        nc.sync.dma_start(out=iota_sb, in_=iota)
        with nc.sync.register("spgate") as greg:
            nc.sync.load(greg, hst0.bitcast(i32)[0:1, 0:1])
        whi_sb = const.tile([P, HT, E], bf16)
        d_whi_a = nc.sync.dma_start(out=whi_sb[:, 0:HT // 2],
                                    in_=whi[:, 0:HT // 2])
        d_whi_b = nc.sync.dma_start(out=whi_sb[:, HT // 2:HT],
                                    in_=whi[:, HT // 2:HT])
        wlo_sb = const.tile([P, HT, E], bf16)
        d_wlo = nc.sync.dma_start(out=wlo_sb, in_=wlo)

        strip_names = []
        gated_dmas = []
        pe_drop = []             # trs + lo-TTs: drop redundant PE waits
        wacc = const.tile([P, ST, TOP_K], f32)
        iacc = const.tile([P, ST, TOP_K], i32)
        junk = const.tile([P, 8], f32)
        junka = const.tile([P, 8], f32)
        junkai = const.tile([P, 8], i32)
        junk_sp = const.tile([P, 32], i32)

        # DVE observes the bias/iota DMA lanes once
        tch_b = nc.vector.tensor_copy(junk[0:1, 0:1], bias_sb[0:1, 0:1])
        tch_i = nc.vector.tensor_copy(junk[0:1, 1:2], iota_sb[0:1, 0:1])
        strip_names.append(tch_b.ins.name)
        strip_names.append(tch_i.ins.name)

        # PE observes the iden DMA lane early; whi/wlo lanes are absorbed
        # just before the first matmul sweep that needs them (so PE never
        # blocks on the weight DMAs before the transposes).
        dummy_ps = dps.tile([1, 1], f32)
        dmy_id = nc.tensor.matmul(dummy_ps, iden_sb[0:1, 0:1],
                                  iden_sb[0:1, 0:1], start=True, stop=True)

        out_dmas = []
        hst = [None] * ST        # fp32 DMA tiles
        hiT = [None] * ST        # bf16 transposed hi tiles [P, HT, P]
        loT = [None] * ST        # bf16 transposed lo tiles
        hi_cp = {}               # (s, c) -> ACT hi-copy inst
        lo_tt = {}               # (s, c) -> DVE lo-TT inst
        logits_ps = [None] * ST
        last_mm = {}             # s -> stop matmul
        last_act = last_dve = None
        last_iacc = [None] * ST

        def mm_block(s):
            lp = lps.tile([P, E], f32, tag="logits")
            logits_ps[s] = lp
            n = 0
            for a_t, w_t in ((hiT[s], whi_sb), (loT[s], whi_sb),
                             (hiT[s], wlo_sb)):
                if s == 0:
                    dmy = nc.tensor.matmul(dummy_ps, w_t[0:1, 0, 0:1],
                                           w_t[0:1, 0, 0:1],
                                           start=True, stop=True)
                for k in range(HT):
                    if s == 0 and n == HT // 2:
                        # absorb the second whi half's DMA lane mid-sweep-1
                        nc.tensor.matmul(dummy_ps, w_t[0:1, HT // 2, 0:1],
                                         w_t[0:1, HT // 2, 0:1],
                                         start=True, stop=True)
                    mm = nc.tensor.matmul(
                        lp, a_t[:, k], w_t[:, k],
                        start=(n == 0), stop=(n == 3 * HT - 1))
                    n += 1
            last_mm[s] = mm
            return mm

        def epilogue(s):
            nonlocal last_act, last_dve
            # gate: ACT absorbs the DVE epilogue progress (scores WAR) before
            # sigmoid; sigmoid then waits only on PE (stop matmul).
            t_act2 = None
            if s > 0:
                t_act2 = nc.scalar.copy(junkai[0:1, 0:1], iacc[0:1, s - 1, 0:1])
                strip_names.append(t_act2.ins.name)
            scores = sco.tile([P, E], f32, tag="scores")
            act = nc.scalar.activation(scores, logits_ps[s],
                                       mybir.ActivationFunctionType.Sigmoid)
            _dep(act, t_act2, sync=False)
            strip_names.append(act.ins.name)
            last_act = act
            t_sc = nc.vector.tensor_copy(junk[0:1, 4:5], scores[0:1, 0:1])
            strip_names.append(t_sc.ins.name)
            sfc = sco.tile([P, E], f32, tag="sfc")
            a1 = nc.vector.tensor_add(sfc, scores, bias_sb)
            _dep(a1, t_sc, sync=False)
            if s == 0:
                _dep(a1, tch_b, sync=False)

            grp = sco.tile([P, G, 8], f32, tag="grp")
            for g in range(G):
                nc.vector.max(out=grp[:, g], in_=sfc[:, g * GS:(g + 1) * GS])
            gsum = sco.tile([P, G], f32, tag="gsum")
            nc.vector.tensor_add(gsum, grp[:, :, 0], grp[:, :, 1])
            g8 = sco.tile([P, 8], f32, tag="g8")
            nc.vector.max(out=g8, in_=gsum)
            gmask = sco.tile([P, G], f32, tag="gmask")
            nc.vector.tensor_tensor(
                out=gmask, in0=gsum,
                in1=g8[:, TOPK_GROUP - 1:TOPK_GROUP].to_broadcast([P, G]),
                op=mybir.AluOpType.is_ge)
            emask = sco.tile([P, G, GS], f32, tag="emask")
            nc.vector.tensor_copy(emask,
                                  gmask.rearrange("p (g one) -> p g one", one=1)
                                  .to_broadcast([P, G, GS]))
            tmp = sco.tile([P, E], f32, tag="tmp")
            nc.vector.tensor_mul(tmp, sfc, emask.rearrange("p g s -> p (g s)"))

            t8 = sco.tile([P, TOP_K], f32, tag="t8")
            ti8 = sco.tile([P, TOP_K], u32, tag="ti8")
            nc.vector.max(out=t8, in_=tmp)
            nc.vector.max_index(out=ti8, in_max=t8, in_values=tmp)

            ti8f = sco.tile([P, TOP_K], f32, tag="ti8f")
            nc.vector.tensor_copy(ti8f, ti8)
            wk = sco.tile([P, TOP_K], f32, tag="wk")
            eqk = sco.tile([P, E], f32, tag="eqk")
            for k in range(TOP_K):
                e1 = nc.vector.scalar_tensor_tensor(
                    out=eqk, in0=iota_sb, scalar=ti8f[:, k:k + 1], in1=scores,
                    op0=mybir.AluOpType.is_equal, op1=mybir.AluOpType.mult,
                    accum_out=wk[:, k:k + 1])
                if s == 0 and k == 0:
                    _dep(e1, tch_i, sync=False)

            denom = sco.tile([P, 1], f32, tag="denom")
            nc.vector.tensor_reduce(denom, wk, axis=mybir.AxisListType.X,
                                    op=mybir.AluOpType.add)
            nc.vector.tensor_scalar_add(denom, denom, EPS)
            rcp = sco.tile([P, 1], f32, tag="rcp")
            nc.vector.reciprocal(rcp, denom)
            nc.vector.tensor_scalar(out=wacc[:, s], in0=wk, scalar1=rcp,
                                    scalar2=ROUTED_SCALING,
                                    op0=mybir.AluOpType.mult,
                                    op1=mybir.AluOpType.mult)
            lic = nc.vector.tensor_copy(iacc[:, s], ti8)
            last_iacc[s] = lic
            last_dve = lic
            # per-tile output DMAs on the SP ring, gated by register loads
            # observing the DVE results; waits stripped to lane-only.
            with nc.sync.register(f"sph{s}") as hreg:
                l1 = nc.sync.load(hreg, wacc.bitcast(i32)[0:1, s, 0:1])
                l2 = nc.sync.load(hreg, iacc[0:1, s, 0:1])
            dw = nc.sync.dma_start(
                out=o_w.rearrange("(t p) k -> p t k", p=P)[:, s], in_=wacc[:, s])
            di = nc.sync.dma_start(
                out=o_i.rearrange("(t p) k -> p t k", p=P)[:, s], in_=iacc[:, s])
            for dd in (dw, di):
                _dep(dd, l1, sync=False)
                _dep(dd, l2, sync=False)
                gated_dmas.append(dd.ins.name)
            out_dmas.append(dw)
            out_dmas.append(di)

        for s in range(ST):
            # hs tile DMA on the ACT HWDGE ring. Issue-order gates the WAR
            # (previous ACT instr waited on tr(s-1) which is after tr(s-2),
            # the last reader of this buffer); waits stripped to lane-only.
            if s == 0:
                t = hst0
            else:
                t = hstp.tile([P, H], f32, tag="hst")
                d_hs = nc.scalar.dma_start(out=t, in_=hs[s * P:(s + 1) * P, :])
                if s >= 2:
                    gated_dmas.append(d_hs.ins.name)
            hst[s] = t

            if s >= 1:
                mm_block(s - 1)
                epilogue(s - 1)

            # PE: absorb this tile's DMA lane(s), then fp32 transposes,
            # 4 k-tiles per PSUM bank. Tile 0 arrives as two half DMAs: the
            # second half's lane is absorbed just before chunk 7.
            dmy = nc.tensor.matmul(dummy_ps, t[0:1, 0:1], t[0:1, 0:1],
                                   start=True, stop=True)
            if s == 0:
                _dep(dmy, dmy_id, sync=False)
            psT = []
            for c in range(NC):
                if s == 0 and c == NC // 2:
                    nc.tensor.matmul(dummy_ps, t[0:1, H // 2:H // 2 + 1],
                                     t[0:1, 0:1], start=True, stop=True)
                pt = tps.tile([P, KC, P], f32, tag="psT")
                psT.append(pt)
                for j in range(KC):
                    tr = nc.tensor.transpose(
                        pt[:, j], t[:, (c * KC + j) * P:(c * KC + j + 1) * P],
                        iden_sb)
                    pe_drop.append(tr.ins.name)
                    if c == 0 and j == 0:
                        _dep(tr, dmy, sync=False)

            # ACT: one touch absorbs DVE progress (hi-buf WAR vs lo-TT of
            # s-2), then rounding copies PSUM->SBUF bf16
            hi = hTp.tile([P, HT, P], bf16, tag="hiT")
            lo = hTp.tile([P, HT, P], bf16, tag="loT")
            hiT[s], loT[s] = hi, lo
            if s >= 2:
                t_ha = nc.scalar.copy(junka[0:1, 1:2],
                                      loT[s - 1][0:1, HT - 1, 0:1])
                strip_names.append(t_ha.ins.name)
            for c in range(NC):
                hc = nc.scalar.copy(hi[:, c * KC:(c + 1) * KC], psT[c])
                hi_cp[(s, c)] = hc
                strip_names.append(hc.ins.name)
            # DVE: lo = psT - hi (waits the ACT hi-copy, which transitively
            # implies the PE transpose)
            for c in range(NC):
                lt = nc.vector.tensor_tensor(
                    out=lo[:, c * KC:(c + 1) * KC], in0=psT[c],
                    in1=hi[:, c * KC:(c + 1) * KC],
                    op=mybir.AluOpType.subtract)
                lo_tt[(s, c)] = lt
                pe_drop.append(lt.ins.name)

        mm_block(ST - 1)
        epilogue(ST - 1)

        # Output-DMA completion guard: marker DMAs queue behind all output
        # DMAs on the same SP HWDGE ring (FIFO per ring); SP register reads
        # of their SBUF destinations imply the output DMAs fully landed.
        junk_dma = const.tile([P, 2, 4], i32)
        for m in range(2):
            dm = nc.sync.dma_start(out=junk_dma[:, m],
                                   in_=iden.bitcast(i32)[:, 4 * m:4 * m + 4])
            _dep(dm, out_dmas[-2], sync=False)
            _dep(dm, out_dmas[-1], sync=False)
        with nc.sync.register("sptail") as rreg:
            for m in range(2):
                nc.sync.load(rreg, junk_dma[0:1, m, 0:1])
        _funnel(nc, [last_act, last_dve, last_mm[ST - 1]], junk_sp)

    return nc, strip_names, gated_dmas, pe_drop


def strip_engine_waits(nc, names, drop_prefixes):
    """Drop waits on the given engine-drain semaphores for named
    instructions whose remaining wait transitively implies them:
    - transposes' bank-WAR DVE wait (lo-TT) implies the ACT hi-copy and the
      prior PE drain;
    - lo-TTs' ACT wait (hi-copy) implies the PE transpose that produced the
      PSUM operand (the hi-copy waits on exactly that transpose)."""
    n = 0
    for f in nc.m.functions:
        for b in f.blocks:
            for i in b.instructions:
                if i.name not in names:
                    continue
                si = i.sync_info
                if not (si and si.on_wait):
                    continue
                keep = [w for w in si.on_wait
                        if not any(w.ant_name.startswith(p)
                                   for p in drop_prefixes)]
                if len(keep) != len(si.on_wait):
                    si.on_wait = keep
                    n += 1
    return n


def strip_drain_lane_waits(nc):
    """Drop DMA-lane waits from the final SP drain: every output DMA rides
    the SP HWDGE ring ahead of the marker DMAs, whose completion the sptail
    register loads already enforce; input-DMA lanes are quiesced by their
    compute consumers."""
    n = 0
    for f in nc.m.functions:
        for b in f.blocks:
            for i in b.instructions:
                if type(i).__name__ != "InstDrain" or str(i.engine) != "EngineType.SP":
                    continue
                si = i.sync_info
                if not (si and si.on_wait):
                    continue
                keep = [w for w in si.on_wait
                        if not w.ant_name.startswith("DMAHW")]
                if len(keep) != len(si.on_wait):
                    si.on_wait = keep
                    n += 1
    return n


def strip_self_waits(nc, only_names=None):
    """Remove sem-ge waits where the waiting engine is the sole updater of
    the semaphore (same-engine drain guards on named touch instructions)."""
    insts = []
    for f in nc.m.functions:
        for b in f.blocks:
            insts.extend(b.instructions)
    from collections import defaultdict
    upd = defaultdict(set)
    for i in insts:
        si = i.sync_info
        if si and si.on_update:
            for u in si.on_update:
                nm = getattr(u, "ant_name", None)
                if nm:
                    upd[nm].add(str(i.engine))
    n = 0
    for i in insts:
        eligible = (only_names is not None and i.name in only_names) or (
            str(i.engine) == "EngineType.SP"
            and type(i).__name__ in ("InstTensorSave", "InstDrain"))
        if not eligible:
            continue
        si = i.sync_info
        if not (si and si.on_wait):
            continue
        keep = [w for w in si.on_wait
                if not (w.wait_mode == "sem-ge-imm"
                        and upd.get(w.ant_name) == {str(i.engine)})]
        if len(keep) != len(si.on_wait):
            si.on_wait = keep
            n += 1
    return n


def strip_gated_dma_waits(nc, names):
    """For DMAs whose issue is gated by same-engine program order, keep only
    the DMA-lane waits."""
    n = 0
    for f in nc.m.functions:
        for b in f.blocks:
            for i in b.instructions:
                if i.name not in names:
                    continue
                si = i.sync_info
                if not (si and si.on_wait):
                    continue
                own = {getattr(u, "ant_name", "") for u in (si.on_update or [])}
                keep = [w for w in si.on_wait if w.ant_name in own]
                if len(keep) != len(si.on_wait):
                    si.on_wait = keep
                    n += 1
    return n


def validate_single_wait(nc, max_waits=1):
    bad = []
    for f in nc.m.functions:
        for b in f.blocks:
            for i in b.instructions:
                si = i.sync_info
                nw = len(si.on_wait) if si and si.on_wait else 0
                if nw > max_waits:
                    dbg = i.debug
                    loc = f"{dbg.filename}:{dbg.lineno}" if dbg else "?"
                    bad.append((i.name, type(i).__name__, str(i.engine), nw,
                                loc, [w.ant_name for w in si.on_wait]))
    return bad


_NC_CACHE = None


def _get_nc():
    global _NC_CACHE
    if _NC_CACHE is None:
        nc, strip_names, gated_dmas, pe_drop = build_program()
        strip_self_waits(nc, only_names=set(strip_names))
        strip_gated_dma_waits(nc, set(gated_dmas))
        strip_engine_waits(nc, set(pe_drop), ("PE_",))
        strip_drain_lane_waits(nc)
        bad = validate_single_wait(nc)
        if bad:
            raise RuntimeError(f"{len(bad)} multi-wait instructions: {bad[:5]}")
        _NC_CACHE = nc
    return _NC_CACHE


def _prep_inputs(hidden_states, weight, e_score_correction_bias):
    hs = np.ascontiguousarray(np.asarray(hidden_states, dtype=np.float32))
    w = np.asarray(weight, dtype=np.float32)
    b = np.asarray(e_score_correction_bias, dtype=np.float32)
    wt = w.T.astype(np.float32)                       # [H, E]
    w_hi = wt.astype(ml_dtypes.bfloat16)
    w_lo = (wt - w_hi.astype(np.float32)).astype(ml_dtypes.bfloat16)
    whi_r = np.ascontiguousarray(
        w_hi.reshape(HT, P, E).transpose(1, 0, 2))    # [P, HT, E]
    wlo_r = np.ascontiguousarray(
        w_lo.reshape(HT, P, E).transpose(1, 0, 2))
    bias_b = np.ascontiguousarray(np.broadcast_to(b, (P, E)).astype(np.float32))
    iota = np.ascontiguousarray(
        np.broadcast_to(np.arange(E, dtype=np.float32), (P, E)))
    iden = np.eye(P, dtype=np.float32)
    return [
        {"hs": hs[c * SL:(c + 1) * SL], "whi": whi_r, "wlo": wlo_r,
         "bias": bias_b, "iota": iota, "iden": iden}
        for c in range(N_CORES)
    ]


def run(hidden_states, weight, e_score_correction_bias, trace=False):
    nc = _get_nc()
    in_maps = _prep_inputs(hidden_states, weight, e_score_correction_bias)
    res = run_bass_kernel_spmd(nc, in_maps, list(range(N_CORES)), trace=trace)
    w = np.concatenate([r["o_w"] for r in res.results], axis=0)
    i = np.concatenate([r["o_i"] for r in res.results], axis=0).astype(np.int32)
    return (w, i), res


def kernel(hidden_states, weight, e_score_correction_bias):
    (w, i), _ = run(hidden_states, weight, e_score_correction_bias)
    return w, i
